# revision 1
# baseline (speedup 1.0000x reference)
"""MoE gate (DeepSeek-style noaux_tc routing) Trainium2 kernel.

kernel(**inputs) takes the FULL unsharded inputs
  hidden_states [4, 4096, 7168] f32, weight [256, 7168] f32,
  e_score_correction_bias [256] f32
and returns the FULL outputs (topk_idx [16384, 8] int32,
topk_weight [16384, 8] float32), matching the jax reference.

Sharding: data-parallel over the 16384-token axis across 8 NeuronCores
(2048 tokens each); gate weight + bias replicated.

Per core: 128-token tiles stream through; x chunks are PE-transposed
(fp32) so the 7168-dim contraction sits on partitions; the gating GEMM
accumulates logits [128, 256] in PSUM using a 3-term fp32r decomposition
  x@w ~= xr@wr + xr@we + xe@wr
(fp32r = fp32 rounded to 12 mantissa bits, 1 PE cycle/row vs 4 for
plain fp32; the residual split drops only the ~2^-26-relative xe@we
term, so logits match plain fp32 to ~1e-7 and the top-k selection is
exact against the fp32 reference). Routing runs fully on-chip with the
DVE top-8 instructions (max8 / max_index), an index-matched bias gather,
and sum-normalization * 2.5.
"""
import sys
sys.path.insert(0, "/opt/trn_rl_repo")
import numpy as np
import concourse.bass as bass
import concourse.tile as tile
from concourse import bacc, mybir

F32 = mybir.dt.float32
F32R = mybir.dt.float32r
U32 = mybir.dt.uint32
I32 = mybir.dt.int32
AF = mybir.ActivationFunctionType
ALU = mybir.AluOpType
AX = mybir.AxisListType

H = 7168
E = 256
NG = 8          # expert groups
GS = E // NG    # group size (32)
NCH = H // 128  # 56 h-chunks
NQ = 14         # weight load slices (startup pipelining)
QC = NCH // NQ  # chunks per slice
BIG = 1.0e30


def _rnd12(x: np.ndarray) -> np.ndarray:
    """Host-side replica of the HW fp32->fp32r rounding (12 explicit
    mantissa bits, round-to-nearest)."""
    f = x.astype(np.float64)
    m, e = np.frexp(f)
    q = np.round(m * (1 << 13)) / (1 << 13)
    return np.ldexp(q, e).astype(np.float32)


def _build(t_core: int, gemm: str = "fp32", n_devices: int = 8,
          copy_engines=("scalar", "scalar"), gather_engines=("vector",),
          repeat: int = 1):
    """in: x [t_core, H] f32; bias_b [128, E]; iota_b [128, E]; ident
    [128, 128]; weights per mode (wT | wTr | wTr+wTe), all [H, E] f32.
    out: idx_out [t_core, 8] i32, w_out [t_core, 8] f32."""
    assert t_core % 128 == 0
    ntiles = t_core // 128
    nc = bacc.Bacc("TRN2", target_bir_lowering=False, debug=False,
                   num_devices=n_devices)

    x_d = nc.dram_tensor("x", [t_core, H], F32, kind="ExternalInput")
    bias_d = nc.dram_tensor("bias_b", [128, E], F32, kind="ExternalInput")
    iota_d = nc.dram_tensor("iota_b", [128, E], F32, kind="ExternalInput")
    ident_d = nc.dram_tensor("ident", [128, 128], F32, kind="ExternalInput")
    if gemm == "fp32":
        wT_d = nc.dram_tensor("wT", [H, E], F32, kind="ExternalInput")
    if gemm in ("f32r3", "f32r1"):
        wTr_d = nc.dram_tensor("wTr", [H, E], F32, kind="ExternalInput")
    idx_d = nc.dram_tensor("idx_out", [t_core, 8], I32, kind="ExternalOutput")
    w_d = nc.dram_tensor("w_out", [t_core, 8], F32, kind="ExternalOutput")

    def make_eng(name):
        if name == "scalar":
            return lambda out, in_: nc.scalar.copy(out, in_)
        eng = getattr(nc, name)
        return lambda out, in_: eng.tensor_copy(out, in_)

    copy_eng = [make_eng(e) for e in copy_engines]
    gat_eng = [getattr(nc, e) for e in gather_engines]

    with tile.TileContext(nc) as tc:
        with (
            tc.tile_pool(name="const", bufs=1) as constp,
            tc.tile_pool(name="xin", bufs=2) as xin,
            tc.tile_pool(name="xt", bufs=4) as xtp,
            tc.tile_pool(name="route", bufs=2) as rp,
            tc.tile_pool(name="small", bufs=2) as sp,
            tc.tile_pool(name="tps", bufs=5, space="PSUM") as tps,
            tc.tile_pool(name="lps", bufs=3, space="PSUM") as lps,
        ):
            # ---- resident constants (small DMAs on the sync queue) ----
            ident = constp.tile([128, 128], F32)
            nc.sync.dma_start(ident[:], ident_d[:])
            # PE warm-up: the HAM clock-gate keeps PE at 1/2-1/4 rate until
            # ~3-4us of sustained activity; burn that in on the identity
            # during the first x-tile DMA so real transposes run full-rate
            warm = lps.tile([128, E], F32, name="warm", tag="logits")
            for _ in range(24):
                nc.tensor.transpose(warm[:, 0:128], ident[:], ident[:])
            bias_sb = constp.tile([128, E], F32)
            nc.gpsimd.dma_start(bias_sb[:], bias_d[:])
            iota_sb = constp.tile([128, E], F32)
            nc.gpsimd.dma_start(iota_sb[:], iota_d[:])

            # ---- weights: SWDGE queue (gpsimd) so the first x-tile DMA on
            # the sync queue is not stuck behind 15-22 MB of weight loads ----
            wtiles = []

            def declare_weight_f32r(dram):
                base = len(wtiles)
                tiles = [constp.tile([128, QC, E], F32R,
                                     name=f"w_sb{base}_{q}",
                                     tag=f"w_sb{base}_{q}")
                         for q in range(NQ)]
                wtiles.append(tiles)
                dview = dram[:].rearrange("(c p) e -> p c e", p=128)

                def load(q):
                    stage = xin.tile([128, QC * E], F32, tag="wstage")
                    nc.gpsimd.dma_start(
                        stage[:].rearrange("p (c e) -> p c e", e=E),
                        dview[:, q * QC:(q + 1) * QC, :])
                    nc.vector.tensor_copy(
                        tiles[q][:],
                        stage[:].rearrange("p (c e) -> p c e", e=E))
                return tiles, load

            def wslice(tiles, j):
                return tiles[j // QC][:, j % QC, :]

            if gemm == "fp32":
                w_sb = constp.tile([128, NCH, E], F32)
                nc.gpsimd.dma_start(
                    w_sb[:], wT_d[:].rearrange("(c p) e -> p c e", p=128))
            elif gemm == "f32r1":
                wr_tiles, wr_load = declare_weight_f32r(wTr_d)
                for q in range(NQ):
                    wr_load(q)
            else:
                # load wT once; split into (wr, we) on device: the f32r
                # copy rounds to 12 mantissa bits, the subtract leaves the
                # residual (rounded again on write, exact to ~2^-26)
                wr_tiles = [constp.tile([128, QC, E], F32R,
                                        name=f"wr_{q}", tag=f"wr_{q}")
                            for q in range(NQ)]
                we_tiles = [constp.tile([128, QC, E], F32R,
                                        name=f"we_{q}", tag=f"we_{q}")
                            for q in range(NQ)]
                wview = wTr_d[:].rearrange("(c p) e -> p c e", p=128)

                def w_load(q):
                    stage = xin.tile([128, QC * E], F32, tag="wstage",
                                     name=f"wstage_{q}", bufs=3)
                    sview = stage[:].rearrange("p (c e) -> p c e", e=E)
                    # ACT-issued HWDGE ring: fast descriptor gen, separate
                    # FIFO from the x loads on the SP ring
                    nc.scalar.dma_start(
                        sview, wview[:, q * QC:(q + 1) * QC, :])
                    nc.vector.tensor_copy(wr_tiles[q][:], sview)
                    nc.vector.tensor_tensor(
                        we_tiles[q][:], sview,
                        wr_tiles[q][:].bitcast(F32), op=ALU.subtract)

            NB = NCH // 4
            PIPE = 4

            def emit_gemm(i):
                x_t = xin.tile([128, H], F32, tag="x_t", name=f"x_{i}")
                for h4 in range(4):
                    nc.sync.dma_start(
                        x_t[:, H // 4 * h4:H // 4 * (h4 + 1)],
                        x_d[128 * i:128 * (i + 1),
                            H // 4 * h4:H // 4 * (h4 + 1)])

                logits = lps.tile([128, E], F32, name=f"logits_{i}", tag="logits")

                def emit_transpose_and_copy(b):
                    tb = tps.tile([128, 512], F32, name=f"tb_{i}_{b}",
                                  tag="tb")
                    for jj in range(4):
                        j = 4 * b + jj
                        nc.tensor.transpose(
                            tb[:, 128 * jj:128 * (jj + 1)],
                            x_t[:, 128 * j:128 * (j + 1)], ident[:])
                    if gemm == "f32r3":
                        xr = xtp.tile([128, 512], F32R, tag="xTr",
                                      name=f"xr_{i}_{b}")
                        xe = xtp.tile([128, 512], F32R, tag="xTe",
                                      name=f"xe_{i}_{b}")
                        copy_eng[b % len(copy_eng)](xr[:], tb[:])
                        nc.vector.tensor_tensor(
                            xe[:], tb[:], xr[:].bitcast(F32),
                            op=ALU.subtract)
                        return (xr, xe)
                    dt = F32 if gemm == "fp32" else F32R
                    xT = xtp.tile([128, 512], dt, tag="xT",
                                  name=f"xT_{i}_{b}")
                    copy_eng[b % len(copy_eng)](xT[:], tb[:])
                    return (xT,)

                def emit_matmuls(b, bufs):
                    for jj in range(4):
                        j = 4 * b + jj
                        if gemm == "fp32":
                            nc.tensor.matmul(
                                logits[:],
                                bufs[0][:, 128 * jj:128 * (jj + 1)],
                                w_sb[:, j, :],
                                start=(j == 0), stop=(j == NCH - 1))
                        elif gemm == "f32r1":
                            nc.tensor.matmul(
                                logits[:],
                                bufs[0][:, 128 * jj:128 * (jj + 1)],
                                wslice(wr_tiles, j),
                                start=(j == 0), stop=(j == NCH - 1))
                        else:
                            xr, xe = bufs
                            nc.tensor.matmul(
                                logits[:], xr[:, 128 * jj:128 * (jj + 1)],
                                wslice(wr_tiles, j),
                                start=(j == 0), stop=False)
                            nc.tensor.matmul(
                                logits[:], xr[:, 128 * jj:128 * (jj + 1)],
                                wslice(we_tiles, j),
                                start=False, stop=False)
                            nc.tensor.matmul(
                                logits[:], xe[:, 128 * jj:128 * (jj + 1)],
                                wslice(wr_tiles, j), start=False,
                                stop=(j == NCH - 1))

                if i == 0 and gemm == "f32r3":
                    for q in range(PIPE):
                        w_load(q)
                pending = {}
                for b in range(PIPE):
                    pending[b] = emit_transpose_and_copy(b)
                for b in range(NB):
                    if b + PIPE < NB:
                        if i == 0 and gemm == "f32r3":
                            w_load(b + PIPE)
                        pending[b + PIPE] = emit_transpose_and_copy(b + PIPE)
                    emit_matmuls(b, pending.pop(b))
                return logits

            def emit_routing(i, logits):
                scores = rp.tile([128, E], F32, tag="scores",
                                 name=f"scores_{i}")
                nc.scalar.activation(scores[:], logits[:], AF.Sigmoid)
                sfc = rp.tile([128, E], F32, tag="sfc", name=f"sfc_{i}")
                nc.vector.tensor_tensor(sfc[:], scores[:], bias_sb[:],
                                        op=ALU.add)

                g8 = sp.tile([128, 64], F32, tag="g8", name=f"g8_{i}")
                for g in range(NG):
                    nc.vector.max(g8[:, 8 * g:8 * g + 8],
                                  sfc[:, GS * g:GS * (g + 1)])
                gsc = sp.tile([128, NG], F32, tag="gsc", name=f"gsc_{i}")
                nc.vector.tensor_reduce(
                    gsc[:],
                    g8[:].rearrange("p (g i) -> p g i", i=8)[:, :, 0:2],
                    axis=AX.X, op=ALU.add)

                gt8 = sp.tile([128, 8], F32, tag="gt8", name=f"gt8_{i}")
                nc.vector.max(gt8[:], gsc[:])
                pen = sp.tile([128, NG], F32, tag="pen", name=f"pen_{i}")
                nc.vector.tensor_scalar(pen[:], gsc[:], gt8[:, 3:4], -BIG,
                                        op0=ALU.is_lt, op1=ALU.mult)

                masked = rp.tile([128, E], F32, tag="masked",
                                 name=f"masked_{i}")
                for g in range(NG):
                    nc.gpsimd.tensor_scalar_add(
                        masked[:, GS * g:GS * (g + 1)],
                        sfc[:, GS * g:GS * (g + 1)], pen[:, g:g + 1])

                m8 = sp.tile([128, 8], F32, tag="m8", name=f"m8_{i}")
                nc.vector.max(m8[:], masked[:])
                i8 = sp.tile([128, 8], U32, tag="i8", name=f"i8_{i}")
                nc.vector.max_index(i8[:], m8[:], masked[:])

                # w_raw[k] = m8[k] - bias[i8[k]] (index-matched gather)
                i8f = sp.tile([128, 8], F32, tag="i8f", name=f"i8f_{i}")
                nc.vector.tensor_copy(i8f[:], i8[:])
                junk = rp.tile([128, E], F32, tag="junk", name=f"junk_{i}")
                biasg = sp.tile([128, 8], F32, tag="biasg",
                                name=f"biasg_{i}")
                for k in range(8):
                    eng = gat_eng[k % len(gat_eng)]
                    eng.scalar_tensor_tensor(
                        junk[:], iota_sb[:], i8f[:, k:k + 1], bias_sb[:],
                        op0=ALU.is_equal, op1=ALU.mult,
                        accum_out=biasg[:, k:k + 1])

                wraw = sp.tile([128, 8], F32, tag="wraw", name=f"wraw_{i}")
                nc.vector.tensor_tensor(wraw[:], m8[:], biasg[:],
                                        op=ALU.subtract)
                ssum = sp.tile([128, 1], F32, tag="ssum", name=f"ssum_{i}")
                nc.vector.tensor_reduce(ssum[:], wraw[:], axis=AX.X,
                                        op=ALU.add)
                inv = sp.tile([128, 1], F32, tag="inv", name=f"inv_{i}")
                nc.vector.reciprocal(inv[:], ssum[:])
                wout = sp.tile([128, 8], F32, tag="wout", name=f"wout_{i}")
                nc.vector.tensor_scalar(wout[:], wraw[:], inv[:], 2.5,
                                        op0=ALU.mult, op1=ALU.mult)

                nc.sync.dma_start(idx_d[128 * i:128 * (i + 1), :],
                                  i8[:].bitcast(I32))
                nc.sync.dma_start(w_d[128 * i:128 * (i + 1), :], wout[:])

            # defer each tile's routing until the next tile's GEMM ops are
            # emitted, so routing never delays the next tile's PSUM copies
            # on the in-order vector/scalar engines
            def emit_all():
                held = {}
                for i in range(ntiles):
                    held[i] = emit_gemm(i)
                    if i >= 1:
                        emit_routing(i - 1, held.pop(i - 1))
                emit_routing(ntiles - 1, held.pop(ntiles - 1))

            if repeat == 1:
                emit_all()
            else:
                # benchmarking only: loop the whole body on-device so one
                # dispatch runs `repeat` iterations (same data, same outputs)
                with tc.For_i(0, repeat, 1):
                    emit_all()

    nc.compile()
    return nc


def _host_prep_unused(x_full: np.ndarray, weight: np.ndarray, bias: np.ndarray,
              gemm: str, n_cores: int = 8):
    """Shard + prep inputs. Returns in_maps list."""
    T = x_full.shape[0]
    tc = T // n_cores
    wT = np.ascontiguousarray(weight.T)  # [H, E]
    base = {
        "bias_b": np.ascontiguousarray(
            np.broadcast_to(bias[None, :], (128, E))),
        "iota_b": np.ascontiguousarray(
            np.broadcast_to(np.arange(E, dtype=np.float32)[None, :],
                            (128, E))),
        "ident": np.eye(128, dtype=np.float32),
    }
    if gemm == "fp32":
        base["wT"] = wT
    else:
        base["wTr"] = wT if gemm == "f32r3" else rnd12(wT)
    maps = []
    for c in range(n_cores):
        m = dict(base)
        m["x"] = np.ascontiguousarray(x_full[c * tc:(c + 1) * tc])
        maps.append(m)
    return maps


_NC_CACHE = {}
_T_FULL = 16384
_N_CORES = 8
_GEMM = "f32r3"


def kernel(hidden_states, weight, e_score_correction_bias):
    from concourse.bass_utils import run_bass_kernel_spmd

    x = np.ascontiguousarray(
        np.asarray(hidden_states, dtype=np.float32).reshape(_T_FULL, H))
    w = np.asarray(weight, dtype=np.float32)
    bias = np.asarray(e_score_correction_bias, dtype=np.float32)
    t_core = _T_FULL // _N_CORES

    if _GEMM not in _NC_CACHE:
        _NC_CACHE[_GEMM] = _build(t_core, gemm=_GEMM, n_devices=_N_CORES)
    nc = _NC_CACHE[_GEMM]

    base = {
        "wTr": np.ascontiguousarray(w.T),
        "bias_b": np.ascontiguousarray(
            np.broadcast_to(bias[None, :], (128, E))),
        "iota_b": np.ascontiguousarray(
            np.broadcast_to(np.arange(E, dtype=np.float32)[None, :],
                            (128, E))),
        "ident": np.eye(128, dtype=np.float32),
    }
    maps = []
    for c in range(_N_CORES):
        m = dict(base)
        m["x"] = np.ascontiguousarray(x[c * t_core:(c + 1) * t_core])
        maps.append(m)

    br = run_bass_kernel_spmd(nc, maps, list(range(_N_CORES)))
    idx = np.concatenate(
        [br.results[c]["idx_out"] for c in range(_N_CORES)],
        axis=0).astype(np.int32)
    wout = np.concatenate(
        [br.results[c]["w_out"] for c in range(_N_CORES)],
        axis=0).astype(np.float32)
    return idx, wout

